# revision 23
# baseline (speedup 1.0000x reference)
"""Trainium2 Bass kernel for nn_BertLexer (weighted layer mix + ragged segment-mean).

Computation (reference):
    w   = softmax(layer_weights)                       # (L,)
    sub = gamma * einsum('l,lbsf->bsf', w, hidden)     # (B,S,F)
    out[b,w,:] = mean over {s : word_ids[b,s]==w} of sub[b,s,:]   (w >= 1)
    out[b,0,:] = mean over all s of sub[b,s,:]

Strategy (8 NeuronCores, data-parallel over B; memory-bound ~30.4 MB/core):
  - Each core gets B/8 = 4 sentences.
  - Layer mix on DVE with 3 scalar_tensor_tensor ops per 128x1536
    half-sentence via ratio folding over weight-sorted layers
    (a<=b<=c<=d by softmax weight):  t1 = h_a*(w_a/w_d) + h_d ;
    t2 = h_b*(w_b/w_c) + h_c ; sub = t2*(w_c/w_d) + t1, and the
    segment matrix absorbs w_d*gamma.  Half-sentence ops amortize the
    ~150-cycle DVE instruction overhead (measured 1.15us per 128x768 op
    vs 1.76us per 128x1536).
  - Segment mean as an f32r matmul with a host-built per-sentence matrix
    M[s, w-1] = w_d*gamma/count_w for s in word w's span; column 256
    holds w_d*gamma/S for the sentence-mean row (out[b,0]), computed by a
    1-col matmul.  Contraction over s on the TensorEngine, accumulated in
    PSUM over the 4 s-chunks; f32r runs the PE at 1 cycle/row (~1e-4 rel
    err).  Matmuls are ordered weights-outer so each 128-col weight block
    loads once per chunk (3 LDWEIGHTS instead of 6).
  - DMA schedule: h loads (786 KB half-sentences) and the first two M
    matrices are the first instructions issued, alternating between the
    two HWDGE rings; later sentences' loads are issued right after the
    compute that frees their buffers so neither ring's FIFO parks on a
    far-future tile-recycle wait.  PSUM->SBUF copies ride the ACT
    engine; stores are split per 128-row word tile.
"""

import numpy as np

L, B, S, F = 4, 32, 512, 768
W_MAX = 256
NW = W_MAX + 1  # 257
NCORES = 8
NB = B // NCORES  # sentences per core
P = 128
SC = S // P  # s-chunks per sentence
NH = SC // 2  # half-sentences per sentence (2 chunks each)
F2 = 2 * F

_module_cache: dict = {}


def _build_module(r0: float, r1: float, r2: float, col0: float, order):
    import concourse.bacc as bacc
    import concourse.bass as bass
    import concourse.mybir as mybir
    import concourse.tile as tile

    f32 = mybir.dt.float32
    f32r = mybir.dt.float32r  # noqa: F841
    bf16 = mybir.dt.bfloat16
    mult = mybir.AluOpType.mult
    add = mybir.AluOpType.add

    nc = bacc.Bacc(
        "TRN2", target_bir_lowering=False, debug=False, num_devices=NCORES
    )
    hid = nc.dram_tensor("hid", (L, NB, S, F), f32, kind="ExternalInput").ap()
    # mm[b, p, c, w] : segment matrix for s = c*128+p; cols 0..255 are
    # words 1..256 (w_d*gamma/count), col 256 is w_d*gamma/S (sentence mean)
    mm = nc.dram_tensor("mm", (NB, P, SC, NW), bf16, kind="ExternalInput").ap()
    out = nc.dram_tensor("out", (NB, NW, F), f32, kind="ExternalOutput").ap()

    wtiles = [(1, 129), (129, 257)]  # output word-id ranges per 128-row tile
    fsplits = [(0, 384), (384, 768)]

    with tile.TileContext(nc) as tc:
        with (
            tc.tile_pool(name="m", bufs=1) as mpool,
            tc.tile_pool(name="h", bufs=40) as hpool,
            tc.tile_pool(name="t", bufs=4) as tpool,
            tc.tile_pool(name="sub", bufs=3) as spool,
            tc.tile_pool(name="o", bufs=4) as opool,
            tc.tile_pool(name="ps", bufs=8, space=bass.MemorySpace.PSUM) as pspool,
        ):
            hts = {}
            mts = {}
            neng = [0]


            def issue_loads(b, c):
                for l in range(L):
                    ht = hpool.tile([P, F], f32, tag="h", name=f"h{b}_{c}_{l}")
                    eng = nc.sync if neng[0] % 2 == 0 else nc.scalar
                    neng[0] += 1
                    eng.dma_start(ht[:], hid[l, b, c * P : (c + 1) * P, :])
                    hts[b, c, l] = ht

            # prefetch; the first chunk's tiles go first so the DVE
            # pipeline starts as early as possible, then the (single) M
            # matrix DMA, then the rest of the 2.5-sentence runway.
            issue_loads(0, 0)
            mmt = mpool.tile([P, NB, SC, NW], bf16, tag="m", name="mm")
            nc.scalar.dma_start(mmt[:], mm.rearrange("b p c w -> p b c w"))
            issue_loads(0, 1)
            issue_loads(0, 2)
            issue_loads(0, 3)
            for c in range(SC):
                issue_loads(1, c)
            for c in range(2):
                issue_loads(2, c)

            ia, ib, ic, id_ = order
            for b in range(NB):
                ps = {}
                for t in range(len(wtiles)):
                    for fi in range(len(fsplits)):
                        ps[t, fi] = pspool.tile(
                            [P, 384], f32, tag="ps", name=f"ps{b}_{t}_{fi}",
                            bufs=6,
                        )
                psc = {
                    fi: pspool.tile(
                        [1, 384], f32, tag="psc", name=f"psc{b}_{fi}", bufs=2
                    )
                    for fi in range(len(fsplits))
                }
                tail = b == NB - 1
                for c in range(SC):
                    first = c == 0
                    last = c == SC - 1
                    t1 = tpool.tile([P, F], f32, tag="t")
                    t2 = tpool.tile([P, F], f32, tag="t")
                    sub = spool.tile([P, F], bf16, tag="sub")
                    # the last sentence mixes per f-half so the matmuls on
                    # the first half overlap the second half's DVE work
                    # (shortens the post-final-load serial chain)
                    mixsplits = fsplits if tail else [(0, F)]
                    for mf0, mf1 in mixsplits:
                        nc.vector.scalar_tensor_tensor(
                            t1[:, mf0:mf1], hts[b, c, ia][:, mf0:mf1],
                            float(r0), hts[b, c, id_][:, mf0:mf1],
                            op0=mult, op1=add,
                        )
                        nc.vector.scalar_tensor_tensor(
                            t2[:, mf0:mf1], hts[b, c, ib][:, mf0:mf1],
                            float(r1), hts[b, c, ic][:, mf0:mf1],
                            op0=mult, op1=add,
                        )
                        nc.vector.scalar_tensor_tensor(
                            sub[:, mf0:mf1], t2[:, mf0:mf1], float(r2),
                            t1[:, mf0:mf1], op0=mult, op1=add,
                        )
                        for t, (w0, w1) in enumerate(wtiles):
                            for fi, (f0, f1) in enumerate(fsplits):
                                if f0 < mf0 or f1 > mf1:
                                    continue
                                nc.tensor.matmul(
                                    ps[t, fi][0:128, 0 : f1 - f0],
                                    mmt[:, b, c, w0 - 1 : w1 - 1],
                                    sub[:, f0:f1],
                                    start=first,
                                    stop=last,
                                )
                        for fi, (f0, f1) in enumerate(fsplits):
                            if f0 < mf0 or f1 > mf1:
                                continue
                            nc.tensor.matmul(
                                psc[fi][0:1, 0 : f1 - f0],
                                mmt[:, b, c, W_MAX : W_MAX + 1],
                                sub[:, f0:f1],
                                start=first,
                                stop=last,
                            )
                    if b == 0 and c >= 2:
                        issue_loads(2, c)
                    elif b == 1:
                        issue_loads(3, c)
                # drain on ACT: free the psc banks first (bufs=2 -> next
                # sentence's col0 matmuls wait on them), then each word
                # tile's banks, storing as soon as its tile is assembled.
                # The last sentence stores per f-half for an earlier start.
                obc = opool.tile([1, F], f32, tag="oc")
                for fi, (f0, f1) in enumerate(fsplits):
                    nc.scalar.copy(obc[0:1, f0:f1], psc[fi][0:1, :])
                for t, (w0, w1) in enumerate(wtiles):
                    ob = opool.tile([P, F], f32, tag="o")
                    eng = nc.sync if t == 1 else nc.scalar
                    for fi, (f0, f1) in enumerate(fsplits):
                        nc.scalar.copy(ob[:, f0:f1], ps[t, fi][0:128, :])
                        if tail:
                            eng.dma_start(out[b, w0:w1, f0:f1], ob[:, f0:f1])
                    if not tail:
                        eng.dma_start(out[b, w0:w1, :], ob[:])
                nc.scalar.dma_start(out[b, 0:1, :], obc[0:1, :])

    nc.compile()
    return nc


def _prepare(hidden_states, layer_weights, gamma, word_ids):
    """Host-side prep: softmax ratios + per-sentence segment matrix."""
    hidden_states = np.ascontiguousarray(hidden_states, dtype=np.float32)
    lw = np.asarray(layer_weights, dtype=np.float64)
    g = float(np.asarray(gamma, dtype=np.float64).reshape(-1)[0])
    ids = np.asarray(word_ids)

    e = np.exp(lw - lw.max())
    w = e / e.sum()  # softmax, float64
    # pair layers sorted by weight so every folded ratio is <= 1:
    #   sub*w[d] = w[a]h[a] + w[b]h[b] + w[c]h[c] + w[d]h[d]
    order = tuple(int(i) for i in np.argsort(w))
    ia, ib, ic, id_ = order
    r0 = float(w[ia] / w[id_])
    r1 = float(w[ib] / w[ic]) if w[ic] > 0 else 0.0
    r2 = float(w[ic] / w[id_])
    scale = float(w[id_] * g)  # absorbed into M
    col0 = float(np.float32(scale / S))

    counts = np.zeros((B, NW), dtype=np.int64)
    for b in range(B):
        counts[b] = np.bincount(ids[b], minlength=NW)
    recip = np.zeros((B, NW), dtype=np.float64)
    nz = counts > 0
    recip[nz] = scale / counts[nz]
    rcpf = np.where(ids > 0, np.take_along_axis(recip, ids, axis=1), 0.0)

    import ml_dtypes

    mmat = np.zeros((B, S, NW), dtype=np.float32)
    bi, si = np.nonzero(ids > 0)
    mmat[bi, si, ids[bi, si] - 1] = rcpf[bi, si]
    mmat[:, :, W_MAX] = col0
    mmat = mmat.reshape(B, SC, P, NW).transpose(0, 2, 1, 3)  # (B, P, SC, NW)
    mmat = np.ascontiguousarray(mmat.astype(ml_dtypes.bfloat16))

    in_maps = []
    for i in range(NCORES):
        bs = slice(i * NB, (i + 1) * NB)
        in_maps.append(
            {
                "hid": np.ascontiguousarray(hidden_states[:, bs]),
                "mm": np.ascontiguousarray(mmat[bs]),
            }
        )
    return (r0, r1, r2, col0, order), in_maps


def _run(inputs: dict, trace: bool = False):
    from concourse.bass_utils import run_bass_kernel_spmd

    params, in_maps = _prepare(**inputs)
    if params not in _module_cache:
        _module_cache[params] = _build_module(*params)
    nc = _module_cache[params]

    res = run_bass_kernel_spmd(
        nc, in_maps, core_ids=list(range(NCORES)), trace=trace
    )
    out = np.concatenate([r["out"] for r in res.results], axis=0)
    return out, res


def kernel(**inputs) -> np.ndarray:
    out, _ = _run(inputs, trace=False)
    return out


# revision 24
# speedup vs baseline: 1.3376x; 1.3376x over previous
"""Trainium2 Bass kernel for nn_BertLexer (weighted layer mix + ragged segment-mean).

Computation (reference):
    w   = softmax(layer_weights)                       # (L,)
    sub = gamma * einsum('l,lbsf->bsf', w, hidden)     # (B,S,F)
    out[b,w,:] = mean over {s : word_ids[b,s]==w} of sub[b,s,:]   (w >= 1)
    out[b,0,:] = mean over all s of sub[b,s,:]

Strategy (8 NeuronCores, data-parallel over B; memory-bound ~30.4 MB/core):
  - Each core gets B/8 = 4 sentences.
  - Layer mix on DVE with 3 scalar_tensor_tensor ops per 128x1536
    half-sentence via ratio folding over weight-sorted layers
    (a<=b<=c<=d by softmax weight):  t1 = h_a*(w_a/w_d) + h_d ;
    t2 = h_b*(w_b/w_c) + h_c ; sub = t2*(w_c/w_d) + t1, and the
    segment matrix absorbs w_d*gamma.  Half-sentence ops amortize the
    ~150-cycle DVE instruction overhead (measured 1.15us per 128x768 op
    vs 1.76us per 128x1536).
  - Segment mean as an f32r matmul with a host-built per-sentence matrix
    M[s, w-1] = w_d*gamma/count_w for s in word w's span; column 256
    holds w_d*gamma/S for the sentence-mean row (out[b,0]), computed by a
    1-col matmul.  Contraction over s on the TensorEngine, accumulated in
    PSUM over the 4 s-chunks; f32r runs the PE at 1 cycle/row (~1e-4 rel
    err).  Matmuls are ordered weights-outer so each 128-col weight block
    loads once per chunk (3 LDWEIGHTS instead of 6).
  - DMA schedule: h loads (786 KB half-sentences) and the first two M
    matrices are the first instructions issued, alternating between the
    two HWDGE rings; later sentences' loads are issued right after the
    compute that frees their buffers so neither ring's FIFO parks on a
    far-future tile-recycle wait.  PSUM->SBUF copies ride the ACT
    engine; stores are split per 128-row word tile.
"""

import numpy as np

L, B, S, F = 4, 32, 512, 768
W_MAX = 256
NW = W_MAX + 1  # 257
NCORES = 8
NB = B // NCORES  # sentences per core
P = 128
SC = S // P  # s-chunks per sentence
NH = SC // 2  # half-sentences per sentence (2 chunks each)
F2 = 2 * F

_module_cache: dict = {}


def _build_module(r0: float, r1: float, r2: float, col0: float, order):
    import concourse.bacc as bacc
    import concourse.bass as bass
    import concourse.mybir as mybir
    import concourse.tile as tile

    f32 = mybir.dt.float32
    f32r = mybir.dt.float32r  # noqa: F841
    bf16 = mybir.dt.bfloat16
    mult = mybir.AluOpType.mult
    add = mybir.AluOpType.add

    nc = bacc.Bacc(
        "TRN2", target_bir_lowering=False, debug=False, num_devices=NCORES
    )
    hid = nc.dram_tensor("hid", (L, NB, S, F), bf16, kind="ExternalInput").ap()
    # mm[b, p, c, w] : segment matrix for s = c*128+p; cols 0..255 are
    # words 1..256 (w_d*gamma/count), col 256 is w_d*gamma/S (sentence mean)
    mm = nc.dram_tensor("mm", (NB, P, SC, NW), bf16, kind="ExternalInput").ap()
    out = nc.dram_tensor("out", (NB, NW, F), f32, kind="ExternalOutput").ap()

    wtiles = [(1, 129), (129, 257)]  # output word-id ranges per 128-row tile
    fsplits = [(0, 384), (384, 768)]

    with tile.TileContext(nc) as tc:
        with (
            tc.tile_pool(name="m", bufs=1) as mpool,
            tc.tile_pool(name="h", bufs=40) as hpool,
            tc.tile_pool(name="t", bufs=4) as tpool,
            tc.tile_pool(name="sub", bufs=3) as spool,
            tc.tile_pool(name="o", bufs=4) as opool,
            tc.tile_pool(name="ps", bufs=8, space=bass.MemorySpace.PSUM) as pspool,
        ):
            hts = {}
            mts = {}
            neng = [0]


            def issue_loads(b, c):
                for l in range(L):
                    ht = hpool.tile([P, F], bf16, tag="h", name=f"h{b}_{c}_{l}")
                    eng = nc.sync if neng[0] % 2 == 0 else nc.scalar
                    neng[0] += 1
                    eng.dma_start(ht[:], hid[l, b, c * P : (c + 1) * P, :])
                    hts[b, c, l] = ht

            # prefetch; the first chunk's tiles go first so the DVE
            # pipeline starts as early as possible, then the (single) M
            # matrix DMA, then the rest of the 2.5-sentence runway.
            issue_loads(0, 0)
            mmt = mpool.tile([P, NB, SC, NW], bf16, tag="m", name="mm")
            nc.scalar.dma_start(mmt[:], mm.rearrange("b p c w -> p b c w"))
            issue_loads(0, 1)
            issue_loads(0, 2)
            issue_loads(0, 3)
            for c in range(SC):
                issue_loads(1, c)
            for c in range(2):
                issue_loads(2, c)

            ia, ib, ic, id_ = order
            for b in range(NB):
                ps = {}
                for t in range(len(wtiles)):
                    for fi in range(len(fsplits)):
                        ps[t, fi] = pspool.tile(
                            [P, 384], f32, tag="ps", name=f"ps{b}_{t}_{fi}",
                            bufs=6,
                        )
                psc = {
                    fi: pspool.tile(
                        [1, 384], f32, tag="psc", name=f"psc{b}_{fi}", bufs=2
                    )
                    for fi in range(len(fsplits))
                }
                tail = b == NB - 1
                for c in range(SC):
                    first = c == 0
                    last = c == SC - 1
                    t1 = tpool.tile([P, F], bf16, tag="t")
                    t2 = tpool.tile([P, F], bf16, tag="t")
                    sub = spool.tile([P, F], bf16, tag="sub")
                    # the last sentence mixes per f-half so the matmuls on
                    # the first half overlap the second half's DVE work
                    # (shortens the post-final-load serial chain)
                    mixsplits = fsplits if tail else [(0, F)]
                    for mf0, mf1 in mixsplits:
                        nc.vector.scalar_tensor_tensor(
                            t1[:, mf0:mf1], hts[b, c, ia][:, mf0:mf1],
                            float(r0), hts[b, c, id_][:, mf0:mf1],
                            op0=mult, op1=add,
                        )
                        nc.vector.scalar_tensor_tensor(
                            t2[:, mf0:mf1], hts[b, c, ib][:, mf0:mf1],
                            float(r1), hts[b, c, ic][:, mf0:mf1],
                            op0=mult, op1=add,
                        )
                        nc.vector.scalar_tensor_tensor(
                            sub[:, mf0:mf1], t2[:, mf0:mf1], float(r2),
                            t1[:, mf0:mf1], op0=mult, op1=add,
                        )
                        for t, (w0, w1) in enumerate(wtiles):
                            for fi, (f0, f1) in enumerate(fsplits):
                                if f0 < mf0 or f1 > mf1:
                                    continue
                                nc.tensor.matmul(
                                    ps[t, fi][0:128, 0 : f1 - f0],
                                    mmt[:, b, c, w0 - 1 : w1 - 1],
                                    sub[:, f0:f1],
                                    start=first,
                                    stop=last,
                                )
                        for fi, (f0, f1) in enumerate(fsplits):
                            if f0 < mf0 or f1 > mf1:
                                continue
                            nc.tensor.matmul(
                                psc[fi][0:1, 0 : f1 - f0],
                                mmt[:, b, c, W_MAX : W_MAX + 1],
                                sub[:, f0:f1],
                                start=first,
                                stop=last,
                            )
                    if b == 0 and c >= 2:
                        issue_loads(2, c)
                    elif b == 1:
                        issue_loads(3, c)
                # drain on ACT: free the psc banks first (bufs=2 -> next
                # sentence's col0 matmuls wait on them), then each word
                # tile's banks, storing as soon as its tile is assembled.
                # The last sentence stores per f-half for an earlier start.
                obc = opool.tile([1, F], f32, tag="oc")
                for fi, (f0, f1) in enumerate(fsplits):
                    nc.scalar.copy(obc[0:1, f0:f1], psc[fi][0:1, :])
                for t, (w0, w1) in enumerate(wtiles):
                    ob = opool.tile([P, F], f32, tag="o")
                    eng = nc.sync if t == 1 else nc.scalar
                    for fi, (f0, f1) in enumerate(fsplits):
                        nc.scalar.copy(ob[:, f0:f1], ps[t, fi][0:128, :])
                        if tail:
                            eng.dma_start(out[b, w0:w1, f0:f1], ob[:, f0:f1])
                    if not tail:
                        eng.dma_start(out[b, w0:w1, :], ob[:])
                nc.scalar.dma_start(out[b, 0:1, :], obc[0:1, :])

    nc.compile()
    return nc


def _prepare(hidden_states, layer_weights, gamma, word_ids):
    """Host-side prep: softmax ratios + per-sentence segment matrix."""
    import ml_dtypes as _mld

    hidden_states = np.ascontiguousarray(
        np.asarray(hidden_states, dtype=np.float32).astype(_mld.bfloat16)
    )
    lw = np.asarray(layer_weights, dtype=np.float64)
    g = float(np.asarray(gamma, dtype=np.float64).reshape(-1)[0])
    ids = np.asarray(word_ids)

    e = np.exp(lw - lw.max())
    w = e / e.sum()  # softmax, float64
    # pair layers sorted by weight so every folded ratio is <= 1:
    #   sub*w[d] = w[a]h[a] + w[b]h[b] + w[c]h[c] + w[d]h[d]
    order = tuple(int(i) for i in np.argsort(w))
    ia, ib, ic, id_ = order
    r0 = float(w[ia] / w[id_])
    r1 = float(w[ib] / w[ic]) if w[ic] > 0 else 0.0
    r2 = float(w[ic] / w[id_])
    scale = float(w[id_] * g)  # absorbed into M
    col0 = float(np.float32(scale / S))

    counts = np.zeros((B, NW), dtype=np.int64)
    for b in range(B):
        counts[b] = np.bincount(ids[b], minlength=NW)
    recip = np.zeros((B, NW), dtype=np.float64)
    nz = counts > 0
    recip[nz] = scale / counts[nz]
    rcpf = np.where(ids > 0, np.take_along_axis(recip, ids, axis=1), 0.0)

    import ml_dtypes

    mmat = np.zeros((B, S, NW), dtype=np.float32)
    bi, si = np.nonzero(ids > 0)
    mmat[bi, si, ids[bi, si] - 1] = rcpf[bi, si]
    mmat[:, :, W_MAX] = col0
    mmat = mmat.reshape(B, SC, P, NW).transpose(0, 2, 1, 3)  # (B, P, SC, NW)
    mmat = np.ascontiguousarray(mmat.astype(ml_dtypes.bfloat16))

    in_maps = []
    for i in range(NCORES):
        bs = slice(i * NB, (i + 1) * NB)
        in_maps.append(
            {
                "hid": np.ascontiguousarray(hidden_states[:, bs]),
                "mm": np.ascontiguousarray(mmat[bs]),
            }
        )
    return (r0, r1, r2, col0, order), in_maps


def _run(inputs: dict, trace: bool = False):
    from concourse.bass_utils import run_bass_kernel_spmd

    params, in_maps = _prepare(**inputs)
    if params not in _module_cache:
        _module_cache[params] = _build_module(*params)
    nc = _module_cache[params]

    res = run_bass_kernel_spmd(
        nc, in_maps, core_ids=list(range(NCORES)), trace=trace
    )
    out = np.concatenate([r["out"] for r in res.results], axis=0)
    return out, res


def kernel(**inputs) -> np.ndarray:
    out, _ = _run(inputs, trace=False)
    return out


# revision 26
# speedup vs baseline: 1.3505x; 1.0096x over previous
"""Trainium2 Bass kernel for nn_BertLexer (weighted layer mix + ragged segment-mean).

Computation (reference):
    w   = softmax(layer_weights)                       # (L,)
    sub = gamma * einsum('l,lbsf->bsf', w, hidden)     # (B,S,F)
    out[b,w,:] = mean over {s : word_ids[b,s]==w} of sub[b,s,:]   (w >= 1)
    out[b,0,:] = mean over all s of sub[b,s,:]

Strategy (8 NeuronCores, data-parallel over B; memory-bound):
  - Each core gets B/8 = 4 sentences.  All device traffic is bf16
    (hidden states are cast on the host; the 2e-2 tolerance has ~6x
    headroom over the measured 3e-3 error), which halves both the HBM
    read bytes (12.6 MB hid + 1.05 MB segment matrix per core) and the
    DVE mixing time (16-bit 2x mode).  Output stays f32.
  - Layer mix on DVE with 3 scalar_tensor_tensor ops per 128x768 chunk
    via ratio folding over weight-sorted layers (a<=b<=c<=d by softmax
    weight): t1 = h_a*(w_a/w_d) + h_d ; t2 = h_b*(w_b/w_c) + h_c ;
    sub = t2*(w_c/w_d) + t1; the segment matrix absorbs w_d*gamma.
  - Segment mean as a bf16 matmul (1 cycle/row even with a cold PE,
    unlike f32r) with a host-built per-sentence matrix
    M[s, w-1] = w_d*gamma/count_w for s in word w's span; column 256
    holds w_d*gamma/S for the sentence-mean row (out[b,0], a 1-col
    matmul).  Contraction over s on the TensorEngine, f32 PSUM
    accumulation over the 4 s-chunks; weights-outer matmul order so
    each 128-col weight block loads once per chunk.
  - DMA schedule: the first chunk's tiles are the first issued DMAs
    (DVE starts ASAP), then the single M-matrix DMA, then a
    2.5-sentence prefetch runway; later sentences' loads are issued
    right after the compute that frees their buffers, alternating the
    two HWDGE rings (each dma_start costs ~0.7us of sequencer time).
    PSUM->SBUF copies ride the ACT engine, psc banks drain first;
    stores split per 128-row word tile.  The last sentence mixes and
    stores per f-half to shorten the post-final-load serial chain.
"""

import numpy as np

L, B, S, F = 4, 32, 512, 768
W_MAX = 256
NW = W_MAX + 1  # 257
NCORES = 8
NB = B // NCORES  # sentences per core
P = 128
SC = S // P  # s-chunks per sentence
NH = SC // 2  # half-sentences per sentence (2 chunks each)
F2 = 2 * F

_module_cache: dict = {}


def _build_module(r0: float, r1: float, r2: float, col0: float, order):
    import concourse.bacc as bacc
    import concourse.bass as bass
    import concourse.mybir as mybir
    import concourse.tile as tile

    f32 = mybir.dt.float32
    f32r = mybir.dt.float32r  # noqa: F841
    bf16 = mybir.dt.bfloat16
    mult = mybir.AluOpType.mult
    add = mybir.AluOpType.add

    nc = bacc.Bacc(
        "TRN2", target_bir_lowering=False, debug=False, num_devices=NCORES
    )
    hid = nc.dram_tensor("hid", (L, NB, S, F), bf16, kind="ExternalInput").ap()
    # mm[b, p, c, w] : segment matrix for s = c*128+p; cols 0..255 are
    # words 1..256 (w_d*gamma/count), col 256 is w_d*gamma/S (sentence mean)
    mm = nc.dram_tensor("mm", (NB, P, SC, NW), bf16, kind="ExternalInput").ap()
    out = nc.dram_tensor("out", (NB, NW, F), f32, kind="ExternalOutput").ap()

    wtiles = [(1, 129), (129, 257)]  # output word-id ranges per 128-row tile
    fsplits = [(0, 384), (384, 768)]

    with tile.TileContext(nc) as tc:
        with (
            tc.tile_pool(name="m", bufs=1) as mpool,
            tc.tile_pool(name="h", bufs=20) as hpool,
            tc.tile_pool(name="t", bufs=4) as tpool,
            tc.tile_pool(name="sub", bufs=3) as spool,
            tc.tile_pool(name="o", bufs=4) as opool,
            tc.tile_pool(name="ps", bufs=8, space=bass.MemorySpace.PSUM) as pspool,
        ):
            hts = {}
            mts = {}
            neng = [0]


            def issue_loads(b, h):
                # one [P, 2F] tile per (layer, half-sentence), filled by two
                # plain chunk DMAs so the DVE can mix 1536 elems per op
                for l in range(L):
                    ht = hpool.tile([P, F2], bf16, tag="h", name=f"h{b}_{h}_{l}")
                    for j in range(2):
                        c = 2 * h + j
                        eng = nc.sync if neng[0] % 2 == 0 else nc.scalar
                        neng[0] += 1
                        eng.dma_start(
                            ht[:, j * F : (j + 1) * F],
                            hid[l, b, c * P : (c + 1) * P, :],
                        )
                    hts[b, h, l] = ht

            # prefetch; the first half-sentence's tiles go first so the DVE
            # pipeline starts as early as possible, then the (single) M
            # matrix DMA, then the rest of the 2.5-sentence runway.
            issue_loads(0, 0)
            mmt = mpool.tile([P, NB, SC, NW], bf16, tag="m", name="mm")
            nc.scalar.dma_start(mmt[:], mm.rearrange("b p c w -> p b c w"))
            issue_loads(0, 1)
            for h in range(NH):
                issue_loads(1, h)
            issue_loads(2, 0)

            ia, ib, ic, id_ = order
            for b in range(NB):
                ps = {}
                for t in range(len(wtiles)):
                    for fi in range(len(fsplits)):
                        ps[t, fi] = pspool.tile(
                            [P, 384], f32, tag="ps", name=f"ps{b}_{t}_{fi}",
                            bufs=6,
                        )
                psc = {
                    fi: pspool.tile(
                        [1, 384], f32, tag="psc", name=f"psc{b}_{fi}", bufs=2
                    )
                    for fi in range(len(fsplits))
                }
                tail = b == NB - 1
                for h in range(NH):
                    t1 = tpool.tile([P, F2], bf16, tag="t")
                    t2 = tpool.tile([P, F2], bf16, tag="t")
                    sub = spool.tile([P, F2], bf16, tag="sub")
                    # the last sentence mixes per quarter so matmuls on the
                    # earlier slices overlap the remaining DVE work
                    # (shortens the post-final-load serial chain)
                    if tail:
                        units = [
                            (j * F + f0, j * F + f1)
                            for j in range(2)
                            for (f0, f1) in fsplits
                        ]
                    else:
                        units = [(0, F2)]
                    for mf0, mf1 in units:
                        nc.vector.scalar_tensor_tensor(
                            t1[:, mf0:mf1], hts[b, h, ia][:, mf0:mf1],
                            float(r0), hts[b, h, id_][:, mf0:mf1],
                            op0=mult, op1=add,
                        )
                        nc.vector.scalar_tensor_tensor(
                            t2[:, mf0:mf1], hts[b, h, ib][:, mf0:mf1],
                            float(r1), hts[b, h, ic][:, mf0:mf1],
                            op0=mult, op1=add,
                        )
                        nc.vector.scalar_tensor_tensor(
                            sub[:, mf0:mf1], t2[:, mf0:mf1], float(r2),
                            t1[:, mf0:mf1], op0=mult, op1=add,
                        )
                        for j in range(2):
                            c = 2 * h + j
                            first = c == 0
                            last = c == SC - 1
                            for fi, (f0, f1) in enumerate(fsplits):
                                g0, g1 = j * F + f0, j * F + f1
                                if g0 < mf0 or g1 > mf1:
                                    continue
                                for t, (w0, w1) in enumerate(wtiles):
                                    nc.tensor.matmul(
                                        ps[t, fi][0:128, 0 : f1 - f0],
                                        mmt[:, b, c, w0 - 1 : w1 - 1],
                                        sub[:, g0:g1],
                                        start=first,
                                        stop=last,
                                    )
                                nc.tensor.matmul(
                                    psc[fi][0:1, 0 : f1 - f0],
                                    mmt[:, b, c, W_MAX : W_MAX + 1],
                                    sub[:, g0:g1],
                                    start=first,
                                    stop=last,
                                )
                    if b == 0 and h == 0:
                        issue_loads(2, 1)
                    elif b == 0 and h == 1:
                        issue_loads(3, 0)
                    elif b == 1 and h == 0:
                        issue_loads(3, 1)
                # drain on ACT: free the psc banks first (bufs=2 -> next
                # sentence's col0 matmuls wait on them), then each word
                # tile's banks, storing as soon as its tile is assembled.
                # The last sentence stores per f-half for an earlier start.
                obc = opool.tile([1, F], f32, tag="oc")
                for fi, (f0, f1) in enumerate(fsplits):
                    nc.scalar.copy(obc[0:1, f0:f1], psc[fi][0:1, :])
                for t, (w0, w1) in enumerate(wtiles):
                    ob = opool.tile([P, F], f32, tag="o")
                    eng = nc.sync if t == 1 else nc.scalar
                    for fi, (f0, f1) in enumerate(fsplits):
                        nc.scalar.copy(ob[:, f0:f1], ps[t, fi][0:128, :])
                        if tail:
                            eng.dma_start(out[b, w0:w1, f0:f1], ob[:, f0:f1])
                    if not tail:
                        eng.dma_start(out[b, w0:w1, :], ob[:])
                nc.scalar.dma_start(out[b, 0:1, :], obc[0:1, :])

    nc.compile()
    return nc


def _prepare(hidden_states, layer_weights, gamma, word_ids):
    """Host-side prep: softmax ratios + per-sentence segment matrix."""
    import ml_dtypes as _mld

    hidden_states = np.ascontiguousarray(
        np.asarray(hidden_states, dtype=np.float32).astype(_mld.bfloat16)
    )
    lw = np.asarray(layer_weights, dtype=np.float64)
    g = float(np.asarray(gamma, dtype=np.float64).reshape(-1)[0])
    ids = np.asarray(word_ids)

    e = np.exp(lw - lw.max())
    w = e / e.sum()  # softmax, float64
    # pair layers sorted by weight so every folded ratio is <= 1:
    #   sub*w[d] = w[a]h[a] + w[b]h[b] + w[c]h[c] + w[d]h[d]
    order = tuple(int(i) for i in np.argsort(w))
    ia, ib, ic, id_ = order
    r0 = float(w[ia] / w[id_])
    r1 = float(w[ib] / w[ic]) if w[ic] > 0 else 0.0
    r2 = float(w[ic] / w[id_])
    scale = float(w[id_] * g)  # absorbed into M
    col0 = float(np.float32(scale / S))

    counts = np.zeros((B, NW), dtype=np.int64)
    for b in range(B):
        counts[b] = np.bincount(ids[b], minlength=NW)
    recip = np.zeros((B, NW), dtype=np.float64)
    nz = counts > 0
    recip[nz] = scale / counts[nz]
    rcpf = np.where(ids > 0, np.take_along_axis(recip, ids, axis=1), 0.0)

    import ml_dtypes

    mmat = np.zeros((B, S, NW), dtype=np.float32)
    bi, si = np.nonzero(ids > 0)
    mmat[bi, si, ids[bi, si] - 1] = rcpf[bi, si]
    mmat[:, :, W_MAX] = col0
    mmat = mmat.reshape(B, SC, P, NW).transpose(0, 2, 1, 3)  # (B, P, SC, NW)
    mmat = np.ascontiguousarray(mmat.astype(ml_dtypes.bfloat16))

    in_maps = []
    for i in range(NCORES):
        bs = slice(i * NB, (i + 1) * NB)
        in_maps.append(
            {
                "hid": np.ascontiguousarray(hidden_states[:, bs]),
                "mm": np.ascontiguousarray(mmat[bs]),
            }
        )
    return (r0, r1, r2, col0, order), in_maps


def _run(inputs: dict, trace: bool = False):
    from concourse.bass_utils import run_bass_kernel_spmd

    params, in_maps = _prepare(**inputs)
    if params not in _module_cache:
        _module_cache[params] = _build_module(*params)
    nc = _module_cache[params]

    res = run_bass_kernel_spmd(
        nc, in_maps, core_ids=list(range(NCORES)), trace=trace
    )
    out = np.concatenate([r["out"] for r in res.results], axis=0)
    return out, res


def kernel(**inputs) -> np.ndarray:
    out, _ = _run(inputs, trace=False)
    return out
